# revision 14
# baseline (speedup 1.0000x reference)
"""Trainium2 Bass kernel for asymmetric-Finsler GNN message passing.

Strategy (8 NeuronCores, SPMD, no collectives):
  - dst-segment sharding: core c owns output nodes [c*NPC, (c+1)*NPC).
    Host buckets each edge to the core owning its dst and sorts by dst,
    so every core's aggregation is purely local (host sharding plays the
    role of the all-to-all in the hint).
  - Edge gathers run on-device via batched int16 dma_gather from per-core
    deduplicated tables (host builds only index arrays / table slices).
    The Q7 descriptor-generation rate (~8ns/row) is the kernel's pacing
    resource, so there is exactly ONE gather per edge slot:
      row = [h(256) | emb(128) | u_j(128) | beta_arg | pad]  bf16
    where u_j = emb @ W_u and beta_arg = emb . w_beta are filled in by a
    device prepass over the (deduplicated) table.
  - The dst-side embedding x_i is NOT gathered: it is reconstructed with
    a one-hot select matmul from the tile's contiguous 128-row emb slab.
  - segment_sum is a one-hot matmul accumulating into PSUM per 128-node
    dst tile; the hidden GEMM is hoisted AFTER aggregation:
      segsum((h[src] @ W.T + b) * w) == segsum(w*h[src]) @ W.T + segsum(w)*b
  - ACT function-table thrash is avoided by keeping all ACT funcs in the
    'exp_and_others' set except Sqrt, whose uses are grouped.
"""

import contextlib

import numpy as np
import ml_dtypes

import concourse.bass as bass
import concourse.tile as tile
from concourse import bacc, mybir
from concourse.bass_utils import run_bass_kernel_spmd
from concourse.library_config import mlp

BF16 = ml_dtypes.bfloat16

CHUNK_E = 128         # edges per chunk (matmul contraction dim)
ROW = 640             # table row elems (bf16): h|emb|u|barg|pad (+1 if bias)


class Cfg:
    def __init__(self, n_cores=8, nodes_per_core=12500, super_=8, thirds=3,
                 in_dim=256, embed=128, gepi=6):
        self.n_cores = n_cores
        self.npc = nodes_per_core
        self.n_tiles = (nodes_per_core + 127) // 128
        self.padn = self.n_tiles * 128
        self.super = super_
        self.thirds = thirds
        self.gepi = gepi              # superbatches per epilogue group
        self.in_dim = in_dim          # == HIDDEN == 256
        self.embed = embed            # 128


def _wrap16(idx, reps=8):
    S = idx.shape[0]
    assert S % 16 == 0
    w = idx.reshape(-1, 16).T.astype(np.int16)
    return np.tile(w, (reps, 1))


def _preprocess(cfg, h, emb, pos_src, pos_dst, neg_src, neg_dst,
                with_bias=False):
    """Build per-core shards, gather tables, schedules (shared across cores)."""
    E_T = {"pos": (pos_src, pos_dst), "neg": (neg_src, neg_dst)}
    percore = [dict() for _ in range(cfg.n_cores)]
    embo = cfg.in_dim + (1 if with_bias else 0)

    counts = {}
    edges = {}
    for k, (srcA, dstA) in E_T.items():
        counts[k] = np.zeros((cfg.n_cores, cfg.n_tiles), np.int64)
        core_of = dstA // cfg.npc
        for c in range(cfg.n_cores):
            sel = np.nonzero(core_of == c)[0]
            dl = dstA[sel] - c * cfg.npc
            order = np.argsort(dl, kind="stable")
            sel = sel[order]
            dl = dl[order]
            edges[(k, c)] = (srcA[sel], dl)
            counts[k][c] = np.bincount(dl >> 7, minlength=cfg.n_tiles)

    sched = {}
    for k in E_T:
        C_t = np.maximum(1, -(-counts[k].max(axis=0) // CHUNK_E))
        slot_base = np.concatenate([[0], np.cumsum(C_t)[:-1]]) * CHUNK_E
        nch = int(C_t.sum())
        t1 = max(1, -(-nch // cfg.thirds))
        third_of_chunk = np.minimum(np.arange(nch) // t1, cfg.thirds - 1)
        sched[k] = dict(C_t=C_t, slot_base=slot_base, nch=nch, S=nch * CHUNK_E,
                        third=third_of_chunk)

    Umax = {(k, th): 1 for k in E_T for th in range(cfg.thirds)}
    core_slots = {}
    for k in E_T:
        sc = sched[k]
        for c in range(cfg.n_cores):
            src, dl = edges[(k, c)]
            tiles = dl >> 7
            cnt = counts[k][c]
            starts = np.concatenate([[0], np.cumsum(cnt)[:-1]])
            rank = np.arange(len(dl)) - starts[tiles]
            slotpos = sc["slot_base"][tiles] + rank
            S = sc["S"]
            src_slot = np.zeros(S, np.int64)
            dst_rel = np.full(S, -1.0, np.float32)
            src_slot[slotpos] = src
            dst_rel[slotpos] = (dl - tiles * 128).astype(np.float32)
            core_slots[(k, c)] = (src_slot, dst_rel)
            for th in range(cfg.thirds):
                smask = np.repeat(sc["third"] == th, CHUNK_E)
                u = np.unique(src_slot[smask])
                if len(u) > 32000:
                    raise RuntimeError(f"uniq overflow {len(u)}")
                Umax[(k, th)] = max(Umax[(k, th)], len(u))

    for k in E_T:
        sc = sched[k]
        for c in range(cfg.n_cores):
            src_slot, dst_rel = core_slots[(k, c)]
            idx16 = np.zeros(sc["S"], np.int64)
            for th in range(cfg.thirds):
                smask = np.repeat(sc["third"] == th, CHUNK_E)
                u, inv = np.unique(src_slot[smask], return_inverse=True)
                idx16[smask] = inv
                U = Umax[(k, th)]
                tab = np.zeros((U, ROW), BF16)
                tab[: len(u), :cfg.in_dim] = h[u].astype(BF16)
                if with_bias:
                    tab[: len(u), cfg.in_dim] = 1.0
                tab[: len(u), embo:embo + cfg.embed] = emb[u].astype(BF16)
                percore[c][f"tab_{k}_{th}"] = tab
                # transposed emb for the u_j prepass: [128, Uceil]
                Uc = -(-U // 128) * 128
                et = np.zeros((cfg.embed, Uc), BF16)
                et[:, : len(u)] = emb[u].T.astype(BF16)
                percore[c][f"embT_{k}_{th}"] = et
            d = percore[c]
            d[f"idx_src_{k}"] = _wrap16(idx16)
            dr = dst_rel.astype(np.float32)
            d[f"dst_rel_{k}"] = np.ascontiguousarray(dr.reshape(-1, 128).T)
            d[f"dst_rel_row_{k}"] = np.ascontiguousarray(dr[None, :]).astype(BF16)

    for c in range(cfg.n_cores):
        lo, hi = c * cfg.npc, (c + 1) * cfg.npc
        emb_loc = np.zeros((cfg.padn, cfg.embed), BF16)
        emb_loc[: hi - lo] = emb[lo:hi].astype(BF16)
        hT = np.zeros((2, 128, cfg.padn), BF16)
        hT_full = h[lo:hi].T.astype(BF16)
        hT[0, :, : hi - lo] = hT_full[:128]
        hT[1, :, : hi - lo] = hT_full[128:]
        percore[c]["emb_loc"] = emb_loc
        percore[c]["hT_loc"] = hT

    return percore, sched, Umax


def _build_graph(cfg, sched, Umax, with_bias):
    nc = bacc.Bacc("TRN2", target_bir_lowering=False, debug=False,
                   num_devices=cfg.n_cores)
    f32, bf16, i16 = mybir.dt.float32, mybir.dt.bfloat16, mybir.dt.int16

    D = {}
    for k in ("pos", "neg"):
        S, nch = sched[k]["S"], sched[k]["nch"]
        D[f"idx_src_{k}"] = nc.dram_tensor(f"idx_src_{k}", [128, S // 16], i16,
                                           kind="ExternalInput")
        D[f"dst_rel_{k}"] = nc.dram_tensor(f"dst_rel_{k}", [128, nch], f32,
                                           kind="ExternalInput")
        D[f"dst_rel_row_{k}"] = nc.dram_tensor(
            f"dst_rel_row_{k}", [1, S], bf16, kind="ExternalInput")
        for th in range(cfg.thirds):
            U = Umax[(k, th)]
            D[f"tab_{k}_{th}"] = nc.dram_tensor(
                f"tab_{k}_{th}", [U, ROW], bf16, kind="ExternalInput")
            D[f"embT_{k}_{th}"] = nc.dram_tensor(
                f"embT_{k}_{th}", [cfg.embed, -(-U // 128) * 128], bf16,
                kind="ExternalInput")
        D[f"wub_{k}"] = nc.dram_tensor(f"wub_{k}", [cfg.embed, cfg.embed + 1],
                                       bf16, kind="ExternalInput")
        D[f"wt_{k}"] = nc.dram_tensor(f"wt_{k}", [2, 128, cfg.in_dim], bf16,
                                      kind="ExternalInput")
        D[f"alpha_{k}"] = nc.dram_tensor(f"alpha_{k}", [128, 1], f32,
                                         kind="ExternalInput")
        if with_bias:
            D[f"b_{k}"] = nc.dram_tensor(f"b_{k}", [1, cfg.in_dim], bf16,
                                         kind="ExternalInput")
    D["wt_self"] = nc.dram_tensor("wt_self", [2, 128, cfg.in_dim], bf16,
                                  kind="ExternalInput")
    if with_bias:
        D["b_self"] = nc.dram_tensor("b_self", [1, cfg.in_dim], bf16,
                                     kind="ExternalInput")
    D["emb_loc"] = nc.dram_tensor("emb_loc", [cfg.padn, cfg.embed], bf16,
                                  kind="ExternalInput")
    D["hT_loc"] = nc.dram_tensor("hT_loc", [2, 128, cfg.padn], bf16,
                                 kind="ExternalInput")
    D["iota"] = nc.dram_tensor("iota", [128, 128], f32, kind="ExternalInput")
    D["iotaT"] = nc.dram_tensor("iotaT", [128, 128], f32, kind="ExternalInput")
    D["ident"] = nc.dram_tensor("ident", [128, 128], bf16,
                                kind="ExternalInput")
    out_d = nc.dram_tensor("out", [cfg.padn, cfg.in_dim], f32,
                           kind="ExternalOutput")

    with tile.TileContext(nc) as tc:
        _emit(cfg, tc, nc, D, out_d, sched, with_bias)
    nc.compile()
    return nc


def _emit(cfg, tc, nc, D, out_d, sched, with_bias):
    f32, bf16, i16 = mybir.dt.float32, mybir.dt.bfloat16, mybir.dt.int16
    ID, EM = cfg.in_dim, cfg.embed
    embo = ID + (1 if with_bias else 0)
    uo = embo + EM               # u_j offset in row
    bargo = uo + EM              # beta_arg offset
    gwid = ID + (1 if with_bias else 0)
    AOP, AF = mybir.AluOpType, mybir.ActivationFunctionType
    ctx = contextlib.ExitStack()
    with ctx:
        const = ctx.enter_context(tc.tile_pool(name="const", bufs=1))
        idxp = ctx.enter_context(tc.tile_pool(name="idx", bufs=3))
        gsrc = ctx.enter_context(tc.tile_pool(name="gsrc", bufs=cfg.gepi + 3))
        slabp = ctx.enter_context(tc.tile_pool(name="slab", bufs=3))
        work = ctx.enter_context(tc.tile_pool(name="work", bufs=3))
        scal = ctx.enter_context(tc.tile_pool(name="scal", bufs=cfg.gepi + 3))
        oneh = ctx.enter_context(tc.tile_pool(name="oneh", bufs=3))
        gsb = ctx.enter_context(tc.tile_pool(name="gsb", bufs=2))
        outp = ctx.enter_context(tc.tile_pool(name="outs", bufs=2))
        hTp = ctx.enter_context(tc.tile_pool(name="hT", bufs=2))
        prep = ctx.enter_context(tc.tile_pool(name="prep", bufs=3))
        # PSUM banks: xi(1) + mc(2) + gp(1) + gn(1) + po(1) = 6
        p_xi = ctx.enter_context(tc.tile_pool(name="xi", bufs=1, space="PSUM"))
        p_misc = ctx.enter_context(tc.tile_pool(name="mc", bufs=2, space="PSUM"))
        p_g = {
            "pos": ctx.enter_context(tc.tile_pool(name="gp", bufs=1, space="PSUM")),
            "neg": ctx.enter_context(tc.tile_pool(name="gn", bufs=1, space="PSUM")),
        }
        p_out = ctx.enter_context(tc.tile_pool(name="po", bufs=1, space="PSUM"))

        nc.gpsimd.load_library(mlp)

        # ---- constants ----------------------------------------------------
        iota = const.tile([128, 128], f32, tag="iota")
        nc.sync.dma_start(iota[:], D["iota"][:, :])
        iotaT = const.tile([128, 128], f32, tag="iotaT")
        nc.sync.dma_start(iotaT[:], D["iotaT"][:, :])
        ident = const.tile([128, 128], bf16, tag="ident")
        nc.sync.dma_start(ident[:], D["ident"][:, :])
        ones_strip = const.tile([1, 128], bf16, tag="ones1")
        nc.vector.memset(ones_strip[:], 1.0)
        wub, wt, negalpha, drel, drowt, brow = {}, {}, {}, {}, {}, {}
        for k in ("pos", "neg"):
            wub[k] = const.tile([EM, EM + 1], bf16, tag=f"wub{k}",
                                name=f"wub{k}")
            nc.sync.dma_start(wub[k][:], D[f"wub_{k}"][:, :])
            wt[k] = const.tile([128, 2 * ID], bf16, tag=f"wt{k}",
                               name=f"wt{k}")
            for _h in range(2):
                nc.sync.dma_start(wt[k][:, _h * ID:(_h + 1) * ID],
                                  D[f"wt_{k}"][_h, :, :])
            a_raw = const.tile([128, 1], f32, tag=f"ar{k}", name=f"ar{k}")
            nc.sync.dma_start(a_raw[:], D[f"alpha_{k}"][:, :])
            na = const.tile([128, 1], f32, tag=f"na{k}", name=f"na{k}")
            nc.vector.tensor_scalar(out=na[:], in0=a_raw[:], scalar1=0.1,
                                    scalar2=10.0, op0=AOP.max, op1=AOP.min)
            nc.vector.tensor_scalar_mul(na[:], na[:], -1.0)
            negalpha[k] = na
            drel[k] = const.tile([128, sched[k]["nch"]], f32, tag=f"dr{k}",
                                 name=f"dr{k}")
            nc.sync.dma_start(drel[k][:], D[f"dst_rel_{k}"][:, :])
            drowt[k] = D[f"dst_rel_row_{k}"]
            if with_bias:
                brow[k] = const.tile([1, ID], bf16, tag=f"b{k}", name=f"b{k}")
                nc.sync.dma_start(brow[k][:], D[f"b_{k}"][:, :])
        wts = const.tile([128, 2 * ID], bf16, tag="wts")
        for _h in range(2):
            nc.sync.dma_start(wts[:, _h * ID:(_h + 1) * ID],
                              D["wt_self"][_h, :, :])
        if with_bias:
            brow["self"] = const.tile([1, ID], bf16, tag="bs", name="bself")
            nc.sync.dma_start(brow["self"][:], D["b_self"][:, :])

        # ---- prepass: fill u_j|beta_arg into the gather tables ------------
        for k in ("pos", "neg"):
            for th in range(cfg.thirds):
                U = Umax_of(D, k, th)
                Uc = -(-U // 128) * 128
                for r0 in range(0, Uc, 128):
                    w_r = min(128, U - r0)
                    if w_r <= 0:
                        break
                    et = prep.tile([128, 128], bf16, tag="et")
                    nc.sync.dma_start(et[:],
                                      D[f"embT_{k}_{th}"][:, r0:r0 + 128])
                    ubp = p_misc.tile([128, EM + 1], f32, tag="mc")
                    nc.tensor.matmul(out=ubp[:], lhsT=et[:], rhs=wub[k][:],
                                     start=True, stop=True,
                                     skip_group_check=True)
                    ubs = prep.tile([128, EM + 1], bf16, tag="ubs")
                    nc.scalar.activation(ubs[:], ubp[:], AF.Copy)
                    nc.sync.dma_start(
                        D[f"tab_{k}_{th}"][r0:r0 + w_r, uo:bargo + 1],
                        ubs[:w_r, :])

        # ---- superbatch machinery ----------------------------------------
        state = {}
        for k in ("pos", "neg"):
            sc = sched[k]
            bounds = []
            g0 = 0
            nch = sc["nch"]
            while g0 < nch:
                th = sc["third"][g0]
                g1 = min(g0 + cfg.super, nch)
                while sc["third"][g1 - 1] != th:
                    g1 -= 1
                bounds.append((g0, g1, int(th)))
                g0 = g1
            state[k] = dict(bounds=bounds, bi=0, cur=None, curmap={},
                            pend=[])

        def fetch_superbatch(k):
            st = state[k]
            g0, g1, th = st["bounds"][st["bi"]]
            st["bi"] += 1
            B = g1 - g0
            nidx = B * CHUNK_E
            it_s = idxp.tile([128, nidx // 16], i16, tag="its")
            nc.sync.dma_start(it_s[:],
                              D[f"idx_src_{k}"][:, g0 * 8:g0 * 8 + nidx // 16])
            drs = idxp.tile([1, B * 128], bf16, tag="drs")
            nc.sync.dma_start(drs[:],
                              drowt[k][:, g0 * 128:g0 * 128 + B * 128])
            cmb = gsrc.tile([128, B, ROW], bf16, tag="cmb")
            nc.gpsimd.dma_gather(cmb[:], D[f"tab_{k}_{th}"][:, :], it_s[:],
                                 nidx, nidx, ROW)
            e2 = scal.tile([128, B], f32, tag="e2")
            asym = scal.tile([128, B], f32, tag="asym")
            barg = scal.tile([128, B], f32, tag="barg")
            st["cur"] = dict(g0=g0, g1=g1, cmb=cmb, e2=e2, asym=asym,
                             barg=barg, w=None, drs=drs)
            st["pend"].append(st["cur"])
            return st["cur"]

        def flush_group(k):
            """Sqrt (grouped), tanh, exp for all pending superbatches."""
            st = state[k]
            pend, st["pend"] = st["pend"], []
            if not pend:
                return
            eucs = []
            for cur in pend:
                B = cur["g1"] - cur["g0"]
                euc = scal.tile([128, B], f32, tag="euc")
                nc.scalar.activation(euc[:], cur["e2"][:], AF.Sqrt)
                eucs.append(euc)
            for cur, euc in zip(pend, eucs):
                B = cur["g1"] - cur["g0"]
                beta = scal.tile([128, B], f32, tag="beta")
                nc.scalar.activation(beta[:], cur["barg"][:], AF.Tanh)
                dd = scal.tile([128, B], f32, tag="dd")
                nc.vector.tensor_tensor(out=dd[:], in0=beta[:],
                                        in1=cur["asym"][:], op=AOP.mult)
                nc.vector.tensor_tensor(out=dd[:], in0=dd[:], in1=euc[:],
                                        op=AOP.add)
                nc.vector.tensor_scalar(out=dd[:], in0=dd[:], scalar1=0.0,
                                        scalar2=negalpha[k][:, :],
                                        op0=AOP.max, op1=AOP.mult)
                w = scal.tile([128, B], f32, tag="w")
                nc.scalar.activation(w[:], dd[:], AF.Exp)
                cur["w"] = w

        slab_cur = {}

        def get_slab(t):
            if slab_cur.get("t") == t:
                return slab_cur["s"]
            s = slabp.tile([128, EM], bf16, tag="slab")
            nc.sync.dma_start(s[:], D["emb_loc"][t * 128:(t + 1) * 128, :])
            slab_cur["t"] = t
            slab_cur["s"] = s
            return s

        def emit_chunk_phaseA(k, g, t):
            st = state[k]
            if st["cur"] is None or g >= st["cur"]["g1"]:
                if len(st["pend"]) >= cfg.gepi:
                    flush_group(k)
                fetch_superbatch(k)
            cur = st["cur"]
            st["curmap"][g] = cur
            ci = g - cur["g0"]
            xj = cur["cmb"][:, ci, embo:embo + EM]
            uj = cur["cmb"][:, ci, uo:uo + EM]
            # beta_arg: strided copy from the gathered rows
            nc.vector.tensor_copy(cur["barg"][:, ci:ci + 1],
                                  cur["cmb"][:, ci, bargo:bargo + 1])
            # onehotT [t, e]: broadcast dst_rel row across partitions (K=1
            # matmul), then compare against partition-index constant.
            bc = p_misc.tile([128, 128], f32, tag="mc")
            nc.tensor.matmul(
                out=bc[:], lhsT=ones_strip[:],
                rhs=cur["drs"][0:1, ci * 128:(ci + 1) * 128],
                start=True, stop=True, skip_group_check=True)
            ohT = oneh.tile([128, 128], bf16, tag="ohT")
            nc.vector.tensor_tensor(out=ohT[:], in0=iotaT[:], in1=bc[:],
                                    op=AOP.is_equal)
            # xi[e, d] via select matmul from the tile slab
            slab = get_slab(t)
            xip = p_xi.tile([128, EM], f32, tag="xi")
            nc.tensor.matmul(out=xip[:], lhsT=ohT[:], rhs=slab[:],
                             start=True, stop=True, skip_group_check=True)
            diff = work.tile([128, EM], bf16, tag="diff")
            nc.vector.tensor_tensor(out=diff[:], in0=xip[:], in1=xj,
                                    op=AOP.subtract)
            junk = work.tile([128, EM], bf16, tag="junk")
            nc.scalar.activation(junk[:], diff[:], AF.Square,
                                 accum_out=cur["e2"][:, ci:ci + 1])
            prod = work.tile([128, EM], bf16, tag="prod")
            nc.vector.tensor_tensor(out=prod[:], in0=diff[:], in1=uj,
                                    op=AOP.mult)
            nc.vector.tensor_reduce(out=cur["asym"][:, ci:ci + 1],
                                    in_=prod[:], axis=mybir.AxisListType.X,
                                    op=AOP.add)

        def emit_chunk_phaseB(k, g, gpsum, first):
            cur = state[k]["curmap"][g]
            ci = g - cur["g0"]
            woh = oneh.tile([128, 128], bf16, tag="woh")
            nc.vector.tensor_scalar(
                out=woh[:], in0=iota[:], scalar1=drel[k][:, g:g + 1],
                scalar2=cur["w"][:, ci:ci + 1],
                op0=AOP.is_equal, op1=AOP.mult)
            hsrc = cur["cmb"][:, ci, 0:gwid]
            nc.tensor.matmul(out=gpsum[:, 0:gwid], lhsT=woh[:], rhs=hsrc,
                             start=first, stop=False, skip_group_check=True)

        cursorA = {"pos": 0, "neg": 0}
        chunk_of_tile = {}
        tile_of_chunk = {}
        for k in ("pos", "neg"):
            sc = sched[k]
            start = 0
            lst = []
            toc = []
            for t in range(cfg.n_tiles):
                n = int(sc["C_t"][t])
                lst.append((start, start + n))
                toc += [t] * n
                start += n
            chunk_of_tile[k] = lst
            tile_of_chunk[k] = toc

        def ensure_phaseA(k, upto):
            while cursorA[k] < sched[k]["nch"] and cursorA[k] < upto:
                g = cursorA[k]
                emit_chunk_phaseA(k, g, tile_of_chunk[k][g])
                cursorA[k] += 1

        for t in range(cfg.n_tiles):
            gp = {}
            for k in ("pos", "neg"):
                g0, g1 = chunk_of_tile[k][t]
                hi = g1
                for (b0, b1, th) in state[k]["bounds"]:
                    if b0 < g1 <= b1:
                        hi = b1
                        break
                ensure_phaseA(k, hi)
                if any(state[k]["curmap"][g].get("w") is None
                       for g in range(g0, g1)):
                    flush_group(k)
                gpsum = p_g[k].tile([128, gwid], f32, tag=f"g{k}",
                                    name=f"g{k}")
                for g in range(g0, g1):
                    emit_chunk_phaseB(k, g, gpsum, first=(g == g0))
                gp[k] = gpsum

            # ---- tile epilogue -------------------------------------------
            opsum = p_out.tile([128, ID], f32, tag="op")
            first = True
            for k in ("pos", "neg"):
                gsb_t = gsb.tile([128, ID], bf16, tag="gsb")
                nc.scalar.activation(gsb_t[:], gp[k][:, 0:ID], AF.Copy)
                for half in range(2):
                    gT_p = p_misc.tile([128, 128], bf16, tag="mc")
                    nc.tensor.transpose(
                        gT_p[:], gsb_t[:, half * 128:(half + 1) * 128],
                        identity=ident[:])
                    gT = gsb.tile([128, 128], bf16, tag="gT")
                    nc.scalar.activation(gT[:], gT_p[:], AF.Copy)
                    nc.tensor.matmul(out=opsum[:], lhsT=gT[:],
                                     rhs=wt[k][:, half * ID:(half + 1) * ID],
                                     start=first, stop=False,
                                     skip_group_check=True)
                    first = False
            for half in range(2):
                hT_t = hTp.tile([128, 128], bf16, tag="hTt")
                nc.sync.dma_start(hT_t[:],
                                  D["hT_loc"][half, :, t * 128:(t + 1) * 128])
                nc.tensor.matmul(out=opsum[:], lhsT=hT_t[:],
                                 rhs=wts[:, half * ID:(half + 1) * ID],
                                 start=False, stop=False,
                                 skip_group_check=True)
            if with_bias:
                for k in ("pos", "neg"):
                    cmat = gsb.tile([128, 128], bf16, tag="cmat")
                    nc.vector.memset(cmat[:], 0.0)
                    nc.vector.tensor_copy(cmat[:, 0:1], gp[k][:, ID:ID + 1])
                    cT_p = p_misc.tile([128, 128], bf16, tag="mc")
                    nc.tensor.transpose(cT_p[:], cmat[:], identity=ident[:])
                    cTs = gsb.tile([1, 128], bf16, tag="cTs")
                    nc.vector.tensor_copy(cTs[:], cT_p[0:1, :])
                    nc.tensor.matmul(out=opsum[:], lhsT=cTs[:], rhs=brow[k][:],
                                     start=False, stop=False,
                                     skip_group_check=True)
                nc.tensor.matmul(out=opsum[:], lhsT=ones_strip[:],
                                 rhs=brow["self"][:], start=False, stop=False,
                                 skip_group_check=True)
            ost = outp.tile([128, ID], f32, tag="ost")
            nc.scalar.activation(ost[:], opsum[:], AF.Relu)
            nc.sync.dma_start(out_d[t * 128:(t + 1) * 128, :], ost[:])


def Umax_of(D, k, th):
    return D[f"tab_{k}_{th}"].shape[0]


def _make_in_maps(cfg, inputs, percore, with_bias):
    shared = {}
    for k, Wk, wbk, Wuk in (("pos", "W_pos", "w_pos_beta", "W_pos_u"),
                            ("neg", "W_neg", "w_neg_beta", "W_neg_u")):
        W = np.asarray(inputs[Wk], np.float32)
        wub = np.concatenate(
            [np.asarray(inputs[Wuk], np.float32),
             np.asarray(inputs[wbk], np.float32)[:, None]], axis=1)
        shared[f"wub_{k}"] = wub.astype(BF16)
        shared[f"wt_{k}"] = np.ascontiguousarray(
            W.T.reshape(2, 128, cfg.in_dim)).astype(BF16)
        alpha = np.float32(np.asarray(inputs[f"alpha_{k}"]))
        shared[f"alpha_{k}"] = np.full((128, 1), alpha, np.float32)
        if with_bias:
            shared[f"b_{k}"] = np.asarray(
                inputs[f"b_{k}"], np.float32)[None, :].astype(BF16)
    shared["wt_self"] = np.ascontiguousarray(
        np.asarray(inputs["W_self"], np.float32).T.reshape(2, 128, cfg.in_dim)
    ).astype(BF16)
    if with_bias:
        shared["b_self"] = np.asarray(
            inputs["b_self"], np.float32)[None, :].astype(BF16)
    shared["iota"] = np.tile(np.arange(128, dtype=np.float32), (128, 1))
    shared["iotaT"] = np.ascontiguousarray(shared["iota"].T)
    shared["ident"] = np.eye(128, dtype=np.float32).astype(BF16)

    in_maps = []
    for c in range(cfg.n_cores):
        m = dict(percore[c])
        m.update(shared)
        in_maps.append(m)
    return in_maps


def run(cfg, inputs, runner=None, trace=False):
    h = np.asarray(inputs["h"], np.float32)
    emb = np.asarray(inputs["node_embeddings"], np.float32)
    with_bias = any(np.any(np.asarray(inputs[b]) != 0)
                    for b in ("b_pos", "b_neg", "b_self"))
    percore, sched, Umax = _preprocess(
        cfg, h, emb,
        np.asarray(inputs["pos_src"], np.int64),
        np.asarray(inputs["pos_dst"], np.int64),
        np.asarray(inputs["neg_src"], np.int64),
        np.asarray(inputs["neg_dst"], np.int64), with_bias=with_bias)
    in_maps = _make_in_maps(cfg, inputs, percore, with_bias)
    nc = _build_graph(cfg, sched, Umax, with_bias)
    if runner is None:
        res = run_bass_kernel_spmd(nc, in_maps,
                                   core_ids=list(range(cfg.n_cores)),
                                   trace=trace)
        run.last_exec_ns = res.exec_time_ns
        run.last_res = res
        outs = [res.results[c]["out"] for c in range(cfg.n_cores)]
    else:
        outs = runner(nc, in_maps)
    n = cfg.n_cores * cfg.npc
    out = np.empty((n, cfg.in_dim), np.float32)
    for c in range(cfg.n_cores):
        out[c * cfg.npc:(c + 1) * cfg.npc] = outs[c][:cfg.npc]
    return out


def kernel(**inputs):
    return run(Cfg(), inputs)


# revision 22
# speedup vs baseline: 1.3211x; 1.3211x over previous
"""Trainium2 Bass kernel for asymmetric-Finsler GNN message passing.

Strategy (8 NeuronCores, SPMD, no collectives):
  - dst-segment sharding: core c owns output nodes [c*NPC, (c+1)*NPC).
    Host buckets each edge to the core owning its dst and sorts by dst,
    so every core's aggregation is purely local (host sharding plays the
    role of the all-to-all in the hint).
  - Edge gathers run on-device via batched int16 dma_gather from per-core
    deduplicated tables (host builds only index arrays / table slices).
    The Q7 descriptor-generation rate (~8ns/row) is the kernel's pacing
    resource, so there is exactly ONE gather per edge slot:
      row = [h(256) | emb(128) | u_j(128) | beta_arg | pad]  bf16
    where u_j = emb @ W_u and beta_arg = emb . w_beta are filled in by a
    device prepass over the (deduplicated) table.
  - The dst-side embedding x_i is NOT gathered: it is reconstructed with
    a one-hot select matmul from the tile's contiguous 128-row emb slab.
  - segment_sum is a one-hot matmul accumulating into PSUM per 128-node
    dst tile; the hidden GEMM is hoisted AFTER aggregation:
      segsum((h[src] @ W.T + b) * w) == segsum(w*h[src]) @ W.T + segsum(w)*b
  - ACT function-table thrash is avoided by keeping all ACT funcs in the
    'exp_and_others' set except Sqrt, whose uses are grouped.
"""

import contextlib

import numpy as np
import ml_dtypes

import concourse.bass as bass
import concourse.tile as tile
from concourse import bacc, mybir
from concourse.bass_utils import run_bass_kernel_spmd
from concourse.library_config import mlp

BF16 = ml_dtypes.bfloat16

CHUNK_E = 128         # edges per chunk (matmul contraction dim)
ROW = 640             # table row elems (bf16): h|emb|u|barg|pad (+1 if bias)


class Cfg:
    def __init__(self, n_cores=8, nodes_per_core=12500, super_=8, thirds=3,
                 in_dim=256, embed=128, gepi=6):
        self.n_cores = n_cores
        self.npc = nodes_per_core
        self.n_tiles = (nodes_per_core + 127) // 128
        self.padn = self.n_tiles * 128
        self.super = super_
        self.thirds = thirds
        self.gepi = gepi              # superbatches per epilogue group
        self.in_dim = in_dim          # == HIDDEN == 256
        self.embed = embed            # 128


def _wrap16(idx, reps=8):
    S = idx.shape[0]
    assert S % 16 == 0
    w = idx.reshape(-1, 16).T.astype(np.int16)
    return np.tile(w, (reps, 1))


def _preprocess(cfg, h, emb, pos_src, pos_dst, neg_src, neg_dst,
                with_bias=False):
    """Build per-core shards, gather tables, schedules (shared across cores)."""
    E_T = {"pos": (pos_src, pos_dst), "neg": (neg_src, neg_dst)}
    percore = [dict() for _ in range(cfg.n_cores)]
    embo = cfg.in_dim + (1 if with_bias else 0)

    counts = {}
    edges = {}
    for k, (srcA, dstA) in E_T.items():
        counts[k] = np.zeros((cfg.n_cores, cfg.n_tiles), np.int64)
        core_of = dstA // cfg.npc
        for c in range(cfg.n_cores):
            sel = np.nonzero(core_of == c)[0]
            dl = dstA[sel] - c * cfg.npc
            order = np.argsort(dl, kind="stable")
            sel = sel[order]
            dl = dl[order]
            edges[(k, c)] = (srcA[sel], dl)
            counts[k][c] = np.bincount(dl >> 7, minlength=cfg.n_tiles)

    sched = {}
    for k in E_T:
        C_t = np.maximum(1, -(-counts[k].max(axis=0) // CHUNK_E))
        slot_base = np.concatenate([[0], np.cumsum(C_t)[:-1]]) * CHUNK_E
        nch = int(C_t.sum())
        t1 = max(1, -(-nch // cfg.thirds))
        third_of_chunk = np.minimum(np.arange(nch) // t1, cfg.thirds - 1)
        sched[k] = dict(C_t=C_t, slot_base=slot_base, nch=nch, S=nch * CHUNK_E,
                        third=third_of_chunk)

    Umax = {(k, th): 1 for k in E_T for th in range(cfg.thirds)}
    core_slots = {}
    for k in E_T:
        sc = sched[k]
        for c in range(cfg.n_cores):
            src, dl = edges[(k, c)]
            tiles = dl >> 7
            cnt = counts[k][c]
            starts = np.concatenate([[0], np.cumsum(cnt)[:-1]])
            rank = np.arange(len(dl)) - starts[tiles]
            slotpos = sc["slot_base"][tiles] + rank
            S = sc["S"]
            src_slot = np.zeros(S, np.int64)
            dst_rel = np.full(S, -1.0, np.float32)
            src_slot[slotpos] = src
            dst_rel[slotpos] = (dl - tiles * 128).astype(np.float32)
            core_slots[(k, c)] = (src_slot, dst_rel)
            for th in range(cfg.thirds):
                smask = np.repeat(sc["third"] == th, CHUNK_E)
                u = np.unique(src_slot[smask])
                if len(u) > 32000:
                    raise RuntimeError(f"uniq overflow {len(u)}")
                Umax[(k, th)] = max(Umax[(k, th)], len(u))

    for k in E_T:
        sc = sched[k]
        for c in range(cfg.n_cores):
            src_slot, dst_rel = core_slots[(k, c)]
            idx16 = np.zeros(sc["S"], np.int64)
            for th in range(cfg.thirds):
                smask = np.repeat(sc["third"] == th, CHUNK_E)
                u, inv = np.unique(src_slot[smask], return_inverse=True)
                idx16[smask] = inv
                U = Umax[(k, th)]
                tab = np.zeros((U, ROW), BF16)
                tab[: len(u), :cfg.in_dim] = h[u].astype(BF16)
                if with_bias:
                    tab[: len(u), cfg.in_dim] = 1.0
                tab[: len(u), embo:embo + cfg.embed] = emb[u].astype(BF16)
                percore[c][f"tab_{k}_{th}"] = tab
                # transposed emb for the u_j prepass: [128, Uceil]
                Uc = -(-U // 128) * 128
                et = np.zeros((cfg.embed, Uc), BF16)
                et[:, : len(u)] = emb[u].T.astype(BF16)
                percore[c][f"embT_{k}_{th}"] = et
            d = percore[c]
            d[f"idx_src_{k}"] = _wrap16(idx16)
            dr = dst_rel.astype(np.float32)
            d[f"dst_rel_{k}"] = np.ascontiguousarray(dr.reshape(-1, 128).T)
            d[f"dst_rel_row_{k}"] = np.ascontiguousarray(dr[None, :]).astype(BF16)

    for c in range(cfg.n_cores):
        lo, hi = c * cfg.npc, (c + 1) * cfg.npc
        emb_loc = np.zeros((cfg.padn, cfg.embed), BF16)
        emb_loc[: hi - lo] = emb[lo:hi].astype(BF16)
        hT = np.zeros((2, 128, cfg.padn), BF16)
        hT_full = h[lo:hi].T.astype(BF16)
        hT[0, :, : hi - lo] = hT_full[:128]
        hT[1, :, : hi - lo] = hT_full[128:]
        percore[c]["emb_loc"] = emb_loc
        percore[c]["hT_loc"] = hT

    return percore, sched, Umax


def _build_graph(cfg, sched, Umax, with_bias):
    nc = bacc.Bacc("TRN2", target_bir_lowering=False, debug=False,
                   num_devices=cfg.n_cores)
    f32, bf16, i16 = mybir.dt.float32, mybir.dt.bfloat16, mybir.dt.int16

    D = {}
    for k in ("pos", "neg"):
        S, nch = sched[k]["S"], sched[k]["nch"]
        D[f"idx_src_{k}"] = nc.dram_tensor(f"idx_src_{k}", [128, S // 16], i16,
                                           kind="ExternalInput")
        D[f"dst_rel_{k}"] = nc.dram_tensor(f"dst_rel_{k}", [128, nch], f32,
                                           kind="ExternalInput")
        D[f"dst_rel_row_{k}"] = nc.dram_tensor(
            f"dst_rel_row_{k}", [1, S], bf16, kind="ExternalInput")
        for th in range(cfg.thirds):
            U = Umax[(k, th)]
            D[f"tab_{k}_{th}"] = nc.dram_tensor(
                f"tab_{k}_{th}", [U, ROW], bf16, kind="ExternalInput")
            D[f"embT_{k}_{th}"] = nc.dram_tensor(
                f"embT_{k}_{th}", [cfg.embed, -(-U // 128) * 128], bf16,
                kind="ExternalInput")
        D[f"wub_{k}"] = nc.dram_tensor(f"wub_{k}", [cfg.embed, cfg.embed + 1],
                                       bf16, kind="ExternalInput")
        D[f"wt_{k}"] = nc.dram_tensor(f"wt_{k}", [2, 128, cfg.in_dim], bf16,
                                      kind="ExternalInput")
        D[f"alpha_{k}"] = nc.dram_tensor(f"alpha_{k}", [128, 1], f32,
                                         kind="ExternalInput")
        if with_bias:
            D[f"b_{k}"] = nc.dram_tensor(f"b_{k}", [1, cfg.in_dim], bf16,
                                         kind="ExternalInput")
    D["wt_self"] = nc.dram_tensor("wt_self", [2, 128, cfg.in_dim], bf16,
                                  kind="ExternalInput")
    if with_bias:
        D["b_self"] = nc.dram_tensor("b_self", [1, cfg.in_dim], bf16,
                                     kind="ExternalInput")
    D["emb_loc"] = nc.dram_tensor("emb_loc", [cfg.padn, cfg.embed], bf16,
                                  kind="ExternalInput")
    D["hT_loc"] = nc.dram_tensor("hT_loc", [2, 128, cfg.padn], bf16,
                                 kind="ExternalInput")
    D["iota"] = nc.dram_tensor("iota", [128, 128], f32, kind="ExternalInput")
    D["iotaT"] = nc.dram_tensor("iotaT", [128, 128], f32, kind="ExternalInput")
    D["ident"] = nc.dram_tensor("ident", [128, 128], bf16,
                                kind="ExternalInput")
    out_d = nc.dram_tensor("out", [cfg.padn, cfg.in_dim], f32,
                           kind="ExternalOutput")

    with tile.TileContext(nc) as tc:
        _emit(cfg, tc, nc, D, out_d, sched, with_bias)
    nc.compile()
    return nc


def _emit(cfg, tc, nc, D, out_d, sched, with_bias):
    f32, bf16, i16 = mybir.dt.float32, mybir.dt.bfloat16, mybir.dt.int16
    ID, EM = cfg.in_dim, cfg.embed
    embo = ID + (1 if with_bias else 0)
    uo = embo + EM               # u_j offset in row
    bargo = uo + EM              # beta_arg offset
    gwid = ID + (1 if with_bias else 0)
    AOP, AF = mybir.AluOpType, mybir.ActivationFunctionType
    ctx = contextlib.ExitStack()
    with ctx:
        const = ctx.enter_context(tc.tile_pool(name="const", bufs=1))
        idxp = ctx.enter_context(tc.tile_pool(name="idx", bufs=3))
        gsrc = ctx.enter_context(tc.tile_pool(name="gsrc", bufs=cfg.gepi + 3))
        slabp = ctx.enter_context(tc.tile_pool(name="slab", bufs=3))
        work = ctx.enter_context(tc.tile_pool(name="work", bufs=6))
        scal = ctx.enter_context(tc.tile_pool(name="scal", bufs=cfg.gepi + 3))
        oneh = ctx.enter_context(tc.tile_pool(name="oneh", bufs=6))
        gsb = ctx.enter_context(tc.tile_pool(name="gsb", bufs=2))
        outp = ctx.enter_context(tc.tile_pool(name="outs", bufs=2))
        hTp = ctx.enter_context(tc.tile_pool(name="hT", bufs=2))
        prep = ctx.enter_context(tc.tile_pool(name="prep", bufs=3))
        # PSUM banks: xi(1) + mc(2) + gp(1) + gn(1) + po(1) = 6
        p_xi = ctx.enter_context(tc.tile_pool(name="xi", bufs=2, space="PSUM"))
        p_misc = ctx.enter_context(tc.tile_pool(name="mc", bufs=2, space="PSUM"))
        p_g = {
            "pos": ctx.enter_context(tc.tile_pool(name="gp", bufs=1, space="PSUM")),
            "neg": ctx.enter_context(tc.tile_pool(name="gn", bufs=1, space="PSUM")),
        }
        p_out = ctx.enter_context(tc.tile_pool(name="po", bufs=1, space="PSUM"))

        nc.gpsimd.load_library(mlp)

        # ---- constants ----------------------------------------------------
        iota = const.tile([128, 128], f32, tag="iota")
        nc.sync.dma_start(iota[:], D["iota"][:, :])
        iotaT = const.tile([128, 128], f32, tag="iotaT")
        nc.sync.dma_start(iotaT[:], D["iotaT"][:, :])
        ident = const.tile([128, 128], bf16, tag="ident")
        nc.sync.dma_start(ident[:], D["ident"][:, :])
        ones_strip = const.tile([1, 128], bf16, tag="ones1")
        nc.vector.memset(ones_strip[:], 1.0)
        nident = const.tile([128, 128], bf16, tag="nident")
        magic = const.tile([128, 128], mybir.dt.int32, tag="magic")
        nc.vector.memset(magic[:], 0x5F3759DF)
        nc.vector.tensor_scalar_mul(nident[:], ident[:], -1.0)
        wub, wt, negalpha, drel, drowt, brow = {}, {}, {}, {}, {}, {}
        for k in ("pos", "neg"):
            wub[k] = const.tile([EM, EM + 1], bf16, tag=f"wub{k}",
                                name=f"wub{k}")
            nc.sync.dma_start(wub[k][:], D[f"wub_{k}"][:, :])
            wt[k] = const.tile([128, 2 * ID], bf16, tag=f"wt{k}",
                               name=f"wt{k}")
            for _h in range(2):
                nc.sync.dma_start(wt[k][:, _h * ID:(_h + 1) * ID],
                                  D[f"wt_{k}"][_h, :, :])
            a_raw = const.tile([128, 1], f32, tag=f"ar{k}", name=f"ar{k}")
            nc.sync.dma_start(a_raw[:], D[f"alpha_{k}"][:, :])
            na = const.tile([128, 1], f32, tag=f"na{k}", name=f"na{k}")
            nc.vector.tensor_scalar(out=na[:], in0=a_raw[:], scalar1=0.1,
                                    scalar2=10.0, op0=AOP.max, op1=AOP.min)
            nc.vector.tensor_scalar_mul(na[:], na[:], -1.0)
            negalpha[k] = na
            drel[k] = const.tile([128, sched[k]["nch"]], f32, tag=f"dr{k}",
                                 name=f"dr{k}")
            nc.sync.dma_start(drel[k][:], D[f"dst_rel_{k}"][:, :])
            drowt[k] = D[f"dst_rel_row_{k}"]
            if with_bias:
                brow[k] = const.tile([1, ID], bf16, tag=f"b{k}", name=f"b{k}")
                nc.sync.dma_start(brow[k][:], D[f"b_{k}"][:, :])
        wts = const.tile([128, 2 * ID], bf16, tag="wts")
        for _h in range(2):
            nc.sync.dma_start(wts[:, _h * ID:(_h + 1) * ID],
                              D["wt_self"][_h, :, :])
        if with_bias:
            brow["self"] = const.tile([1, ID], bf16, tag="bs", name="bself")
            nc.sync.dma_start(brow["self"][:], D["b_self"][:, :])

        # ---- prepass: fill u_j|beta_arg into the gather tables ------------
        for k in ("pos", "neg"):
            for th in range(cfg.thirds):
                U = Umax_of(D, k, th)
                Uc = -(-U // 128) * 128
                for r0 in range(0, Uc, 128):
                    w_r = min(128, U - r0)
                    if w_r <= 0:
                        break
                    et = prep.tile([128, 128], bf16, tag="et")
                    nc.sync.dma_start(et[:],
                                      D[f"embT_{k}_{th}"][:, r0:r0 + 128])
                    ubp = p_misc.tile([128, EM + 1], f32, tag="mc")
                    nc.tensor.matmul(out=ubp[:], lhsT=et[:], rhs=wub[k][:],
                                     start=True, stop=True,
                                     skip_group_check=True)
                    ubs = prep.tile([128, EM + 2], bf16, tag="ubs")
                    nc.scalar.activation(ubs[:, 0:EM + 1], ubp[:], AF.Copy)
                    pj = prep.tile([128, EM], bf16, tag="pj")
                    suu = prep.tile([128, 1], f32, tag="suu")
                    nc.scalar.activation(pj[:], ubs[:, 0:EM], AF.Square,
                                         accum_out=suu[:, :])
                    nc.vector.tensor_copy(ubs[:, EM + 1:EM + 2], suu[:, :])
                    nc.sync.dma_start(
                        D[f"tab_{k}_{th}"][r0:r0 + w_r, uo:bargo + 2],
                        ubs[:w_r, :])

        # ---- superbatch machinery ----------------------------------------
        state = {}
        for k in ("pos", "neg"):
            sc = sched[k]
            bounds = []
            g0 = 0
            nch = sc["nch"]
            while g0 < nch:
                th = sc["third"][g0]
                g1 = min(g0 + cfg.super, nch)
                while sc["third"][g1 - 1] != th:
                    g1 -= 1
                bounds.append((g0, g1, int(th)))
                g0 = g1
            state[k] = dict(bounds=bounds, bi=0, cur=None, curmap={},
                            pend=[])

        def fetch_superbatch(k):
            st = state[k]
            g0, g1, th = st["bounds"][st["bi"]]
            st["bi"] += 1
            B = g1 - g0
            nidx = B * CHUNK_E
            it_s = idxp.tile([128, nidx // 16], i16, tag="its")
            nc.sync.dma_start(it_s[:],
                              D[f"idx_src_{k}"][:, g0 * 8:g0 * 8 + nidx // 16])
            drs = idxp.tile([128, B * 128], bf16, tag="drs")
            nc.sync.dma_start(
                drs[:], drowt[k][:, g0 * 128:g0 * 128 + B * 128]
                .to_broadcast([128, B * 128]))
            cmb = gsrc.tile([128, B, ROW], bf16, tag="cmb")
            nc.gpsimd.dma_gather(cmb[:], D[f"tab_{k}_{th}"][:, :], it_s[:],
                                 nidx, nidx, ROW)
            if st.get("gbuf") is None:
                gcap = cfg.gepi * cfg.super
                st["gbuf"] = dict(
                    e2=scal.tile([128, gcap], f32, tag="e2", name="e2g"),
                    s2=scal.tile([128, gcap], f32, tag="s2", name="s2g"),
                    suu=scal.tile([128, gcap], f32, tag="suu2", name="suug"),
                    beta=scal.tile([128, gcap], f32, tag="beta", name="betag"),
                    w=scal.tile([128, gcap], f32, tag="w", name="wg"),
                    used=0)
            gb = st["gbuf"]
            o = gb["used"]
            gb["used"] += B
            st["cur"] = dict(g0=g0, g1=g1, cmb=cmb, e2=gb["e2"][:, o:o + B],
                             s2=gb["s2"][:, o:o + B],
                             suu=gb["suu"][:, o:o + B],
                             beta=gb["beta"][:, o:o + B],
                             w=None, wslice=(gb, o), drs=drs)
            st["pend"].append(st["cur"])
            return st["cur"]

        def flush_group(k):
            """Batched rsqrt-NR, tanh, exp over the whole group."""
            st = state[k]
            pend, st["pend"] = st["pend"], []
            if not pend:
                return
            gb = pend[0]["wslice"][0]
            U = gb["used"]
            e2 = gb["e2"][:, 0:U]
            w = gb["w"][:, 0:U]
            # per-superbatch strided reads from the gathered rows
            for cur in pend:
                B = cur["g1"] - cur["g0"]
                nc.scalar.activation(cur["beta"][:],
                                     cur["cmb"][:, 0:B, bargo], AF.Tanh)
                nc.scalar.activation(cur["suu"][:],
                                     cur["cmb"][:, 0:B, bargo + 1],
                                     AF.Identity)
            asym = scal.tile([128, U], f32, tag="asymc", name="asymc")
            nc.vector.tensor_tensor(out=asym[:], in0=gb["s2"][:, 0:U],
                                    in1=e2, op=AOP.subtract)
            nc.vector.tensor_tensor(out=asym[:], in0=asym[:],
                                    in1=gb["suu"][:, 0:U], op=AOP.subtract)
            asym = asym[:]
            beta = gb["beta"][:, 0:U]
            euc = scal.tile([128, U], f32, tag="euc")
            r = scal.tile([128, U], f32, tag="rsq")
            tmp = scal.tile([128, U], f32, tag="rtmp")
            x = scal.tile([128, U], f32, tag="xcl")
            nc.vector.tensor_scalar_max(x[:], e2, 1e-12)
            nc.vector.tensor_scalar(out=r[:].bitcast(mybir.dt.int32),
                                    in0=x[:].bitcast(mybir.dt.int32),
                                    scalar1=1, scalar2=None,
                                    op0=AOP.arith_shift_right)
            nc.vector.tensor_tensor(out=r[:].bitcast(mybir.dt.int32),
                                    in0=magic[:, 0:U],
                                    in1=r[:].bitcast(mybir.dt.int32),
                                    op=AOP.subtract)
            for _ in range(2):
                nc.vector.tensor_tensor(out=tmp[:], in0=r[:], in1=r[:],
                                        op=AOP.mult)
                nc.vector.tensor_tensor(out=tmp[:], in0=tmp[:],
                                        in1=x[:], op=AOP.mult)
                nc.vector.tensor_scalar(out=tmp[:], in0=tmp[:],
                                        scalar1=-0.5, scalar2=1.5,
                                        op0=AOP.mult, op1=AOP.add)
                nc.vector.tensor_tensor(out=r[:], in0=r[:], in1=tmp[:],
                                        op=AOP.mult)
            nc.vector.tensor_tensor(out=euc[:], in0=x[:], in1=r[:],
                                    op=AOP.mult)
            dd = scal.tile([128, U], f32, tag="dd")
            nc.vector.tensor_tensor(out=dd[:], in0=beta, in1=asym,
                                    op=AOP.mult)
            nc.vector.tensor_scalar_mul(dd[:], dd[:], 0.5)
            nc.vector.tensor_tensor(out=dd[:], in0=dd[:], in1=euc[:],
                                    op=AOP.add)
            nc.vector.tensor_scalar(out=dd[:], in0=dd[:], scalar1=0.0,
                                    scalar2=negalpha[k][:, :],
                                    op0=AOP.max, op1=AOP.mult)
            nc.scalar.activation(w, dd[:], AF.Exp)
            for cur in pend:
                gb2, o = cur["wslice"]
                B = cur["g1"] - cur["g0"]
                cur["w"] = gb2["w"][:, o:o + B]
            st["gbuf"] = None

        slab_cur = {}

        def get_slab(t):
            if slab_cur.get("t") == t:
                return slab_cur["s"]
            s = slabp.tile([128, EM], bf16, tag="slab")
            nc.sync.dma_start(s[:], D["emb_loc"][t * 128:(t + 1) * 128, :])
            slab_cur["t"] = t
            slab_cur["s"] = s
            return s

        def emit_chunk_phaseA(k, g, t):
            st = state[k]
            if st["cur"] is None or g >= st["cur"]["g1"]:
                if len(st["pend"]) >= cfg.gepi:
                    flush_group(k)
                fetch_superbatch(k)
            cur = st["cur"]
            st["curmap"][g] = cur
            ci = g - cur["g0"]
            xj = cur["cmb"][:, ci, embo:embo + EM]
            uj = cur["cmb"][:, ci, uo:uo + EM]
            # onehotT [t, e] = (t == dst_rel[e]) against DMA-broadcast rows
            ohT = oneh.tile([128, 128], bf16, tag="ohT")
            nc.vector.tensor_tensor(
                out=ohT[:], in0=iotaT[:],
                in1=cur["drs"][:, ci * 128:(ci + 1) * 128], op=AOP.is_equal)
            # diff[e, d] = xi - xj computed entirely on PE:
            # xi via select matmul from the tile slab, then -I @ xj.
            slab = get_slab(t)
            xip = p_xi.tile([128, EM], f32, tag="xi")
            nc.tensor.matmul(out=xip[:], lhsT=ohT[:], rhs=slab[:],
                             start=True, stop=False, skip_group_check=True)
            nc.tensor.matmul(out=xip[:], lhsT=nident[:], rhs=xj,
                             start=False, stop=False, skip_group_check=True)
            junk = work.tile([128, EM], bf16, tag="junk")
            nc.scalar.activation(junk[:], xip[:], AF.Square,
                                 accum_out=cur["e2"][:, ci:ci + 1])
            nc.tensor.matmul(out=xip[:], lhsT=ident[:], rhs=uj,
                             start=False, stop=True, skip_group_check=True)
            junk2 = work.tile([128, EM], bf16, tag="junk2")
            nc.scalar.activation(junk2[:], xip[:], AF.Square,
                                 accum_out=cur["s2"][:, ci:ci + 1])

        def emit_chunk_phaseB(k, g, gpsum, first):
            cur = state[k]["curmap"][g]
            ci = g - cur["g0"]
            woh = oneh.tile([128, 128], bf16, tag="woh")
            nc.vector.tensor_scalar(
                out=woh[:], in0=iota[:], scalar1=drel[k][:, g:g + 1],
                scalar2=cur["w"][:, ci:ci + 1],
                op0=AOP.is_equal, op1=AOP.mult)
            hsrc = cur["cmb"][:, ci, 0:gwid]
            nc.tensor.matmul(out=gpsum[:, 0:gwid], lhsT=woh[:], rhs=hsrc,
                             start=first, stop=False, skip_group_check=True)

        cursorA = {"pos": 0, "neg": 0}
        chunk_of_tile = {}
        tile_of_chunk = {}
        for k in ("pos", "neg"):
            sc = sched[k]
            start = 0
            lst = []
            toc = []
            for t in range(cfg.n_tiles):
                n = int(sc["C_t"][t])
                lst.append((start, start + n))
                toc += [t] * n
                start += n
            chunk_of_tile[k] = lst
            tile_of_chunk[k] = toc

        def ensure_phaseA(k, upto):
            while cursorA[k] < sched[k]["nch"] and cursorA[k] < upto:
                g = cursorA[k]
                emit_chunk_phaseA(k, g, tile_of_chunk[k][g])
                cursorA[k] += 1

        for t in range(cfg.n_tiles):
            gp = {}
            for k in ("pos", "neg"):
                g0, g1 = chunk_of_tile[k][t]
                hi = g1
                for (b0, b1, th) in state[k]["bounds"]:
                    if b0 < g1 <= b1:
                        hi = b1
                        break
                ensure_phaseA(k, hi)
                if any(state[k]["curmap"][g].get("w") is None
                       for g in range(g0, g1)):
                    flush_group(k)
                gpsum = p_g[k].tile([128, gwid], f32, tag=f"g{k}",
                                    name=f"g{k}")
                for g in range(g0, g1):
                    emit_chunk_phaseB(k, g, gpsum, first=(g == g0))
                gp[k] = gpsum

            # ---- tile epilogue -------------------------------------------
            opsum = p_out.tile([128, ID], f32, tag="op")
            first = True
            for k in ("pos", "neg"):
                gsb_t = gsb.tile([128, ID], bf16, tag="gsb")
                nc.scalar.activation(gsb_t[:], gp[k][:, 0:ID], AF.Copy)
                for half in range(2):
                    gT_p = p_misc.tile([128, 128], bf16, tag="mc")
                    nc.tensor.transpose(
                        gT_p[:], gsb_t[:, half * 128:(half + 1) * 128],
                        identity=ident[:])
                    gT = gsb.tile([128, 128], bf16, tag="gT")
                    nc.scalar.activation(gT[:], gT_p[:], AF.Copy)
                    nc.tensor.matmul(out=opsum[:], lhsT=gT[:],
                                     rhs=wt[k][:, half * ID:(half + 1) * ID],
                                     start=first, stop=False,
                                     skip_group_check=True)
                    first = False
            for half in range(2):
                hT_t = hTp.tile([128, 128], bf16, tag="hTt")
                nc.sync.dma_start(hT_t[:],
                                  D["hT_loc"][half, :, t * 128:(t + 1) * 128])
                nc.tensor.matmul(out=opsum[:], lhsT=hT_t[:],
                                 rhs=wts[:, half * ID:(half + 1) * ID],
                                 start=False, stop=False,
                                 skip_group_check=True)
            if with_bias:
                for k in ("pos", "neg"):
                    cmat = gsb.tile([128, 128], bf16, tag="cmat")
                    nc.vector.memset(cmat[:], 0.0)
                    nc.vector.tensor_copy(cmat[:, 0:1], gp[k][:, ID:ID + 1])
                    cT_p = p_misc.tile([128, 128], bf16, tag="mc")
                    nc.tensor.transpose(cT_p[:], cmat[:], identity=ident[:])
                    cTs = gsb.tile([1, 128], bf16, tag="cTs")
                    nc.vector.tensor_copy(cTs[:], cT_p[0:1, :])
                    nc.tensor.matmul(out=opsum[:], lhsT=cTs[:], rhs=brow[k][:],
                                     start=False, stop=False,
                                     skip_group_check=True)
                nc.tensor.matmul(out=opsum[:], lhsT=ones_strip[:],
                                 rhs=brow["self"][:], start=False, stop=False,
                                 skip_group_check=True)
            ost = outp.tile([128, ID], f32, tag="ost")
            nc.scalar.activation(ost[:], opsum[:], AF.Relu)
            nc.sync.dma_start(out_d[t * 128:(t + 1) * 128, :], ost[:])


def Umax_of(D, k, th):
    return D[f"tab_{k}_{th}"].shape[0]


def _make_in_maps(cfg, inputs, percore, with_bias):
    shared = {}
    for k, Wk, wbk, Wuk in (("pos", "W_pos", "w_pos_beta", "W_pos_u"),
                            ("neg", "W_neg", "w_neg_beta", "W_neg_u")):
        W = np.asarray(inputs[Wk], np.float32)
        wub = np.concatenate(
            [np.asarray(inputs[Wuk], np.float32),
             np.asarray(inputs[wbk], np.float32)[:, None]], axis=1)
        shared[f"wub_{k}"] = wub.astype(BF16)
        shared[f"wt_{k}"] = np.ascontiguousarray(
            W.T.reshape(2, 128, cfg.in_dim)).astype(BF16)
        alpha = np.float32(np.asarray(inputs[f"alpha_{k}"]))
        shared[f"alpha_{k}"] = np.full((128, 1), alpha, np.float32)
        if with_bias:
            shared[f"b_{k}"] = np.asarray(
                inputs[f"b_{k}"], np.float32)[None, :].astype(BF16)
    shared["wt_self"] = np.ascontiguousarray(
        np.asarray(inputs["W_self"], np.float32).T.reshape(2, 128, cfg.in_dim)
    ).astype(BF16)
    if with_bias:
        shared["b_self"] = np.asarray(
            inputs["b_self"], np.float32)[None, :].astype(BF16)
    shared["iota"] = np.tile(np.arange(128, dtype=np.float32), (128, 1))
    shared["iotaT"] = np.ascontiguousarray(shared["iota"].T)
    shared["ident"] = np.eye(128, dtype=np.float32).astype(BF16)

    in_maps = []
    for c in range(cfg.n_cores):
        m = dict(percore[c])
        m.update(shared)
        in_maps.append(m)
    return in_maps


def run(cfg, inputs, runner=None, trace=False):
    h = np.asarray(inputs["h"], np.float32)
    emb = np.asarray(inputs["node_embeddings"], np.float32)
    with_bias = any(np.any(np.asarray(inputs[b]) != 0)
                    for b in ("b_pos", "b_neg", "b_self"))
    percore, sched, Umax = _preprocess(
        cfg, h, emb,
        np.asarray(inputs["pos_src"], np.int64),
        np.asarray(inputs["pos_dst"], np.int64),
        np.asarray(inputs["neg_src"], np.int64),
        np.asarray(inputs["neg_dst"], np.int64), with_bias=with_bias)
    in_maps = _make_in_maps(cfg, inputs, percore, with_bias)
    nc = _build_graph(cfg, sched, Umax, with_bias)
    if runner is None:
        res = run_bass_kernel_spmd(nc, in_maps,
                                   core_ids=list(range(cfg.n_cores)),
                                   trace=trace)
        run.last_exec_ns = res.exec_time_ns
        run.last_res = res
        outs = [res.results[c]["out"] for c in range(cfg.n_cores)]
    else:
        outs = runner(nc, in_maps)
    n = cfg.n_cores * cfg.npc
    out = np.empty((n, cfg.in_dim), np.float32)
    for c in range(cfg.n_cores):
        out[c * cfg.npc:(c + 1) * cfg.npc] = outs[c][:cfg.npc]
    return out


def kernel(**inputs):
    return run(Cfg(), inputs)
